# revision 40
# baseline (speedup 1.0000x reference)
"""BEiT-style attention (B=128, N=197, C=768, H=12) on 8 TRN2 NeuronCores.

Strategy: pure data parallelism over batch — each core processes 16
samples end-to-end; no collectives. Host pre-gathers the per-sample
bitfit biases (b_idx lookups), pre-transposes x to [C, N] per sample,
folds the attention scale into the q weights/bias, and pre-computes
exp(rel_pos_bias)^T so softmax(S + rpb) = normalize(exp(S) * exp_rpbT).

Device per sample:
  qkT  [1536,197] = w_qk @ x^T          (weights stationary, 2-sample batch)
  v    [197, 768] = x @ w_v^T + v_bias  (x^T stationary slices; bias folded
  into the PSUM drain via a free-broadcast tensor_add)
  per head: S^T[m,n] = k_h @ q_h^T  (two heads packed in the 128-row PE
  array block-diagonally), P = exp(S^T) * exp_rpbT (exp on Act per m-tile,
  rpb multiply on the Pool engine), out^T[hd+1, n] = [v_h | 1]^T @ P
  (ones column yields the softmax denominator), reciprocal on Act,
  denominator broadcast to 64 partitions via SBUF->SBUF DMA (no PE
  ones-matmul), normalize on DVE,
  y^T [768,197] = w_proj @ out_allT + b_proj (bias folded into the drain).
Host transposes the gathered y^T back to [B, N, C].

k_bias is structurally zero (reference registers it as a zeros buffer),
so the k-half PSUM drain is a plain wide copy.
"""
import numpy as np
import ml_dtypes

import concourse.bass as bass
import concourse.tile as tile
from concourse import mybir
from concourse.bass_utils import run_bass_kernel_spmd

B, N, C = 128, 197, 768
H, HD = 12, 64
NCORES = 8
BL = B // NCORES          # 16 samples per core
NPAIRS = BL // 2          # 8 sample pairs
N2 = 2 * N                # 394
KT = C // 128             # 6 k-tiles
MT_QK = 2 * C // 128      # 12 m-tiles of qkT
BF16 = mybir.dt.bfloat16
F32 = mybir.dt.float32
AF = mybir.ActivationFunctionType
ALU = mybir.AluOpType
PV_TRAIL = 2
RPB_ON_POOL = True


def _split_sync_waits(nc, max_waits=1, max_updates=1):
    """TPB descriptors have ONE wait and ONE update slot; hoist extras
    onto same-engine NoOps (trailing-nop updates are completion-safe)."""
    n_split = 0
    for f in nc.m.functions:
        for bb in f.blocks:
            old = list(bb.instructions)
            new = []
            changed = False
            for ins in old:
                si = ins.sync_info
                tname = type(ins).__name__
                is_dma = ("TensorLoad" in tname or "TensorSave" in tname
                          or "TensorCopy" in tname or "TriggeredCopy" in tname)
                if si is None or is_dma:
                    new.append(ins)
                    continue
                waits = list(si.on_wait)
                updates = list(si.on_update)
                if len(waits) <= max_waits and len(updates) <= max_updates:
                    new.append(ins)
                    continue
                changed = True
                n_split += 1
                while len(waits) > max_waits:
                    w = waits.pop(0)
                    new.append(mybir.InstNoOp(
                        name=nc.get_next_instruction_name(), engine=ins.engine,
                        sync_info=mybir.SyncInfo(on_wait=[w], on_update=[]),
                        bass_nofuse=True))
                post = []
                while len(updates) > max_updates:
                    u = updates.pop()
                    post.append(mybir.InstNoOp(
                        name=nc.get_next_instruction_name(), engine=ins.engine,
                        sync_info=mybir.SyncInfo(on_wait=[], on_update=[u]),
                        bass_nofuse=True))
                ins.sync_info = mybir.SyncInfo(on_wait=waits, on_update=updates)
                new.append(ins)
                new.extend(post)
            if changed:
                bb.instructions = new
    return n_split


def build_nc(repeat=1, split_waits=True):
    nc = bass.Bass("TRN2")
    xt_d = nc.declare_dram_parameter("xt", [BL, 128, KT * N], BF16, isOutput=False)
    wqkv_d = nc.declare_dram_parameter("wqkv", [128, KT, 3 * C], BF16, isOutput=False)
    wproj_d = nc.declare_dram_parameter("wproj", [128, KT, C], BF16, isOutput=False)
    rpb_d = nc.declare_dram_parameter("rpb", [N, H, N], BF16, isOutput=False)
    qkvb_d = nc.declare_dram_parameter("qkvb", [128, MT_QK, BL], F32, isOutput=False)
    projb_d = nc.declare_dram_parameter("projb", [128, KT, BL], F32, isOutput=False)
    vb_d = nc.declare_dram_parameter("vb", [BL, H * HD], BF16, isOutput=False)
    out_d = nc.declare_dram_parameter("out", [BL, C, N], F32, isOutput=True)

    with tile.TileContext(nc) as tc:
        with (
            tc.tile_pool(name="const", bufs=1) as const,
            tc.tile_pool(name="xtp", bufs=3) as xtp,
            tc.tile_pool(name="qkp", bufs=3) as qkp,
            tc.tile_pool(name="vbp", bufs=4) as vbp,
            tc.tile_pool(name="esp", bufs=8) as esp,
            tc.tile_pool(name="rcp", bufs=5) as rcp,
            tc.tile_pool(name="oap", bufs=3) as oap,
            tc.tile_pool(name="yp", bufs=3) as yp,
            tc.tile_pool(name="psA", bufs=3, space="PSUM") as psA,
            tc.tile_pool(name="psS", bufs=3, space="PSUM") as psS,
            tc.tile_pool(name="psB", bufs=2, space="PSUM") as psB,
        ):
            # ---- resident constants ----
            wqkv_sb = const.tile([128, KT, 3 * C], BF16)
            nc.sync.dma_start(wqkv_sb, wqkv_d[:])
            wproj_sb = const.tile([128, KT, C], BF16)
            nc.sync.dma_start(wproj_sb, wproj_d[:])
            # rpb packed [p, mt, h, n]; rows 69:128 of mt=1 unused
            rpb_ext = const.tile([128, 2, H, N], BF16)
            nc.sync.dma_start(rpb_ext[:, 0], rpb_d[0:128])
            nc.sync.dma_start(rpb_ext[0:69, 1], rpb_d[128:N])
            qkvb_sb = const.tile([128, MT_QK, BL], F32)
            nc.sync.dma_start(qkvb_sb, qkvb_d[:])
            projb_sb = const.tile([128, KT, BL], F32)
            nc.sync.dma_start(projb_sb, projb_d[:])

            # Persistent ping-pong tiles (parity = pair index % 2): the
            # block-diagonal zeros of q2 and the per-head ones blocks of v
            # never change, so write them once instead of every pair.
            q2_pp = []
            for par in range(2):
                q2 = const.tile([128, KT, 2, N2], BF16, tag=f"q2p{par}",
                                name=f"q2p{par}")
                nc.gpsimd.memset(q2[0:64, :, :, N:N2], 0.0)
                nc.gpsimd.memset(q2[64:128, :, :, 0:N], 0.0)
                q2_pp.append(q2)
            # v stationary per head: [v_h | ones64] -> PV matmul output rows
            # 0:64 carry v-weighted sums, rows 64:128 carry the softmax
            # denominator replicated 64x (free partition-broadcast on PE)
            vt_pp = []
            for par in range(2):
                by_s = []
                for s in range(2):
                    by_nt = []
                    for nt in range(2):
                        nts = 128 if nt == 0 else N - 128
                        vt = const.tile([nts, H, 2 * HD], BF16,
                                        tag=f"vt{par}{s}{nt}",
                                        name=f"vt{par}{s}{nt}")
                        nc.gpsimd.memset(vt[:, :, HD:2 * HD], 1.0)
                        by_nt.append(vt)
                    by_s.append(by_nt)
                vt_pp.append(by_s)

            if True:
                NG = repeat * NPAIRS
                st = {}

                def emit_load(p):
                    pp = p % NPAIRS
                    sg = (2 * pp, 2 * pp + 1)
                    xt = xtp.tile([128, KT, N2], BF16, name="xt")
                    for s in range(2):
                        nc.sync.dma_start(
                            xt[:, :, s * N:(s + 1) * N],
                            xt_d[sg[s]].rearrange("p (k n) -> p k n", k=KT))
                    vb_bc = [None, None]
                    for s in range(2):
                        vb_bc[s] = vbp.tile([128, H, HD], BF16, tag="vb", name="vb")
                        nc.scalar.dma_start(
                            vb_bc[s],
                            vb_d[sg[s]:sg[s] + 1, :].rearrange(
                                "o (h d) -> o h d", h=H).to_broadcast([128, H, HD]))
                    qkT = qkp.tile([128, KT, N2], BF16, name="qkT")
                    # q block-diagonal [j2, hp, s, (hh n)]: rows 0:64 carry
                    # [q_h1 | 0], rows 64:128 [0 | q_h2], so each score tile
                    # [S_h1 | S_h2] is ONE full-stream ap394 matmul (short
                    # ap197 matmuls pay a ~173ns PE-SBUF latency floor)
                    st[p] = {"xt": xt, "vb": vb_bc, "qkT": qkT,
                             "q2": q2_pp[p % 2], "v": None}

                def emit_qkv_m(p, m):
                    pp = p % NPAIRS
                    sg0 = 2 * pp
                    xt, qkT, q2 = st[p]["xt"], st[p]["qkT"], st[p]["q2"]
                    ps = psA.tile([128, N2], F32, tag="mm", name="ps")
                    for k in range(KT):
                        nc.tensor.matmul(ps, wqkv_sb[:, k, m * 128:(m + 1) * 128],
                                         xt[:, k, :], start=(k == 0),
                                         stop=(k == KT - 1))
                    if m >= KT:
                        # k rows: bias is structurally zero -> one wide copy
                        nc.vector.tensor_copy(qkT[:, m - KT, :], ps)
                    else:
                        # q rows: drain + per-sample bias in one DVE op per
                        # diagonal block (bias broadcast along n)
                        for b in range(2):   # head 2m+b -> diagonal block
                            dst = q2[b * 64:(b + 1) * 64, m, :,
                                     b * N:(b + 1) * N]
                            src_ = ps[b * 64:(b + 1) * 64, :].rearrange(
                                "p (s n) -> p s n", s=2)
                            bb = qkvb_sb[b * 64:(b + 1) * 64, m,
                                         sg0:sg0 + 2].to_broadcast([64, 2, N])
                            nc.vector.tensor_add(dst, src_, bb)

                def emit_v_chunk(p, s, nt):
                    xt, vb_bc = st[p]["xt"], st[p]["vb"]
                    if st[p]["v"] is None:
                        st[p]["v"] = [[None, None], [None, None]]
                    nts = 128 if nt == 0 else N - 128
                    vt = vt_pp[p % 2][s][nt]
                    for half in range(2):
                        ps = psA.tile([128, N2], F32, tag="mm", name="ps")
                        for k in range(KT):
                            nc.tensor.matmul(
                                ps[:nts, 0:384],
                                xt[:, k, s * N + nt * 128:
                                   s * N + nt * 128 + nts],
                                wqkv_sb[:, k, 2 * C + half * 384:
                                        2 * C + (half + 1) * 384],
                                start=(k == 0), stop=(k == KT - 1))
                        # drain + v_bias in one DVE op
                        nc.vector.tensor_add(
                            vt[:, half * 6:(half + 1) * 6, 0:HD],
                            ps[:nts, 0:384].rearrange("p (h d) -> p h d", h=6),
                            vb_bc[s][:nts, half * 6:(half + 1) * 6, :])
                    st[p]["v"][s][nt] = vt

                def emit_v(p):
                    for s in range(2):
                        for nt in range(2):
                            emit_v_chunk(p, s, nt)

                def emit_s(p, s, hp):
                    # merged head-pair score: ONE full-stream ap394 matmul per
                    # m-tile (contraction 128 = stacked head-dims); one PSUM
                    # bank per m-tile so psA can have a third buffer
                    qkT, q2 = st[p]["qkT"], st[p]["q2"]
                    ha = 2 * hp
                    e = esp.tile([128, 2, N2], BF16, tag="es", name="es")
                    for mt in range(2):
                        mts = 128 if mt == 0 else N - 128
                        pss = psS.tile([128, 512], F32, tag="s1", name="pss")
                        nc.tensor.matmul(
                            pss[:mts, 0:N2],
                            qkT[:, hp,
                                s * N + mt * 128: s * N + mt * 128 + mts],
                            q2[:, hp, s, :],
                            start=True, stop=True)
                        nc.scalar.activation(
                            e[0:mts, mt, :], pss[0:mts, 0:N2], AF.Exp)
                        # mt1 rpb on the idle Pool engine; mt0 (bigger, and
                        # first needed by PV) stays on DVE
                        rpb_eng = nc.gpsimd if (RPB_ON_POOL and mt == 1) \
                            else nc.vector
                        rpb_eng.tensor_mul(
                            e[0:mts, mt, :].rearrange("p (b n) -> p b n", b=2),
                            e[0:mts, mt, :].rearrange("p (b n) -> p b n", b=2),
                            rpb_ext[0:mts, mt, ha:ha + 2, :])
                    return e

                def emit_pv(p, s, hp, es, oa):
                    v_sb = st[p]["v"]
                    # stationary [v_h | ones64]: output rows 0:64 = PV sums,
                    # rows 64:128 = softmax denominator replicated 64x (the
                    # broadcast costs nothing on PE; cost = moving free size)
                    pvt = psB.tile([128, N2], F32, tag="pv", name="pvt")
                    for mt in range(2):
                        for hh, h in enumerate((2 * hp, 2 * hp + 1)):
                            # start=True clears has_written for the WHOLE bank:
                            # only the very first matmul may set it.
                            mts = 128 if mt == 0 else N - 128
                            nc.tensor.matmul(
                                pvt[:, hh * N:(hh + 1) * N],
                                v_sb[s][mt][:, h, :],
                                es[0:mts, mt, hh * N:(hh + 1) * N],
                                start=(mt == 0 and hh == 0),
                                stop=(mt == 1 and hh == 1))
                    # normalize: 1/denom via exp(-ln d) on Act -- the DVE
                    # reciprocal instruction measures ~3.1us on real HW for
                    # this shape, vs ~0.45us per Act table op
                    lnd = rcp.tile([64, N2], F32, tag="rc", name="lnd")
                    nc.scalar.activation(lnd[:], pvt[HD:2 * HD, :], AF.Ln)
                    rec_bc = rcp.tile([64, N2], BF16, tag="bc_sb", name="rec_bc")
                    nc.scalar.activation(rec_bc[:], lnd[:], AF.Exp, scale=-1.0)
                    for hh in range(2):
                        nc.vector.tensor_mul(
                            oa[hh * 64:(hh + 1) * 64, hp, s * N:(s + 1) * N],
                            pvt[0:HD, hh * N:(hh + 1) * N],
                            rec_bc[:, hh * N:(hh + 1) * N])

                def emit_proj(p):
                    pp = p % NPAIRS
                    sg = (2 * pp, 2 * pp + 1)
                    oa = st[p]["oa"]
                    for m in range(KT):
                        ps = psA.tile([128, N2], F32, tag="mm", name="ps")
                        for k in range(KT):
                            nc.tensor.matmul(
                                ps, wproj_sb[:, k, m * 128:(m + 1) * 128],
                                oa[:, k, :], start=(k == 0), stop=(k == KT - 1))
                        y = yp.tile([128, N2], F32, tag="y", name="y")
                        nc.vector.tensor_add(
                            y[:].rearrange("p (s n) -> p s n", s=2),
                            ps[:].rearrange("p (s n) -> p s n", s=2),
                            projb_sb[:, m, sg[0]:sg[0] + 2].to_broadcast(
                                [128, 2, N]))
                        for s in range(2):
                            nc.sync.dma_start(
                                out_d[sg[s], m * 128:(m + 1) * 128, :],
                                y[:, s * N:(s + 1) * N])

                # skewed pipeline: pair p attention carries pair p+1 QKV
                emit_load(0)
                for m in range(MT_QK):
                    emit_qkv_m(0, m)
                emit_v(0)
                for p in range(NG):
                    if p + 1 < NG:
                        emit_load(p + 1)
                    oa = oap.tile([128, KT, N2], BF16, name="oa")
                    st[p]["oa"] = oa
                    pending = []
                    iters = [(s, hp) for s in range(2) for hp in range(H // 2)]
                    for i, (s, hp) in enumerate(iters):
                        es = emit_s(p, s, hp)
                        if p + 1 < NG:
                            if i < 8:
                                emit_qkv_m(p + 1, i)
                            else:
                                emit_v_chunk(p + 1, (i - 8) // 2, (i - 8) % 2)
                        pending.append((s, hp, es))
                        if len(pending) > PV_TRAIL:
                            emit_pv(p, *pending.pop(0), oa)
                    for item in pending:
                        emit_pv(p, *item, oa)
                    if p + 1 < NG:
                        for m in range(8, MT_QK):
                            emit_qkv_m(p + 1, m)
                    emit_proj(p)
                    del st[p]
    if split_waits:
        _split_sync_waits(nc)
    return nc


_NC_CACHE = {}


def _get_nc():
    if "nc" not in _NC_CACHE:
        _NC_CACHE["nc"] = build_nc()
    return _NC_CACHE["nc"]


def _prep(x, b_idx, w_qkv, q_bias, k_bias, v_bias, rel_pos_table, rel_index,
          w_proj, b_proj):
    x = np.asarray(x, dtype=np.float32)
    b_idx = np.asarray(b_idx)
    w_qkv = np.asarray(w_qkv, dtype=np.float32)
    q_bias = np.asarray(q_bias, dtype=np.float32)
    k_bias = np.asarray(k_bias, dtype=np.float32)
    v_bias = np.asarray(v_bias, dtype=np.float32)
    rel_pos_table = np.asarray(rel_pos_table, dtype=np.float32)
    rel_index = np.asarray(rel_index)
    w_proj = np.asarray(w_proj, dtype=np.float32)
    b_proj = np.asarray(b_proj, dtype=np.float32)

    scale = HD ** (-0.5)
    # fold attention scale into q weights/bias
    w_all = w_qkv.copy()
    w_all[0:C] *= scale
    wqkvT = np.ascontiguousarray(w_all.T)                      # [C, 3C]
    wqkv_p = wqkvT.reshape(KT, 128, 3 * C).transpose(1, 0, 2)  # [128, KT, 3C]
    wprojT = np.ascontiguousarray(w_proj.T)                    # [C, C]
    wproj_p = wprojT.reshape(KT, 128, C).transpose(1, 0, 2)    # [128, KT, C]

    # per-sample gathered biases
    qk_bias = np.concatenate([q_bias * scale, k_bias], axis=1)[b_idx]  # [B, 2C]
    qkvb_all = qk_bias.T.reshape(MT_QK, 128, B)                # [12, 128, B]
    projb_all = b_proj[b_idx].T.reshape(KT, 128, B)            # [6, 128, B]
    vb_all = v_bias[b_idx]                                     # [B, C]

    # exp of transposed relative-position bias: rpbT[m, h, n] = rpb[h][n, m]
    tbl = rel_pos_table[rel_index.reshape(-1)].reshape(N, N, H)  # [n, m, h]
    rpbT = np.exp(tbl.transpose(1, 2, 0))                        # [m, h, n]
    rpb_p = np.ascontiguousarray(rpbT, dtype=np.float32).astype(ml_dtypes.bfloat16)

    # x^T packed: [B, 128, KT*N] with partition p = c % 128, free (k, n)
    xT = x.transpose(0, 2, 1)                                  # [B, C, N]
    xt_p = xT.reshape(B, KT, 128, N).transpose(0, 2, 1, 3).reshape(B, 128, KT * N)
    xt_p = xt_p.astype(ml_dtypes.bfloat16)

    wqkv_p = np.ascontiguousarray(wqkv_p).astype(ml_dtypes.bfloat16)
    wproj_p = np.ascontiguousarray(wproj_p).astype(ml_dtypes.bfloat16)

    in_maps = []
    for i in range(NCORES):
        lo, hi = i * BL, (i + 1) * BL
        in_maps.append({
            "xt": np.ascontiguousarray(xt_p[lo:hi]),
            "wqkv": wqkv_p,
            "wproj": wproj_p,
            "rpb": rpb_p,
            "qkvb": np.ascontiguousarray(qkvb_all.transpose(1, 0, 2)[:, :, lo:hi]).astype(np.float32),
            "projb": np.ascontiguousarray(projb_all.transpose(1, 0, 2)[:, :, lo:hi]).astype(np.float32),
            "vb": np.ascontiguousarray(vb_all[lo:hi]).astype(ml_dtypes.bfloat16),
        })

    return in_maps


def _gather(results):
    outT = np.concatenate([results[i]["out"] for i in range(NCORES)], axis=0)
    return np.ascontiguousarray(outT.transpose(0, 2, 1))


def kernel(**inputs):
    in_maps = _prep(**inputs)
    nc = _get_nc()
    res = run_bass_kernel_spmd(nc, in_maps, list(range(NCORES))).results
    return _gather(res)



# revision 43
# speedup vs baseline: 1.6920x; 1.6920x over previous
"""BEiT-style attention (B=128, N=197, C=768, H=12) on 8 TRN2 NeuronCores.

Strategy: pure data parallelism over batch — each core processes 16
samples end-to-end; no collectives. Host pre-gathers the per-sample
bitfit biases (b_idx lookups), pre-transposes x to [C, N] per sample,
folds the attention scale into the q weights/bias, and pre-computes
exp(rel_pos_bias)^T so softmax(S + rpb) = normalize(exp(S) * exp_rpbT).

Device per sample:
  qkT  [1536,197] = w_qk @ x^T          (weights stationary, 2-sample batch)
  v    [197, 768] = x @ w_v^T + v_bias  (x^T stationary slices; bias folded
  into the PSUM drain via a free-broadcast tensor_add)
  per head: S^T[m,n] = k_h @ q_h^T  (two heads packed in the 128-row PE
  array block-diagonally), P = exp(S^T) * exp_rpbT (exp on Act per m-tile,
  rpb multiply on the Pool engine), out^T[hd+1, n] = [v_h | 1]^T @ P
  (ones column yields the softmax denominator), reciprocal on Act,
  denominator broadcast to 64 partitions via SBUF->SBUF DMA (no PE
  ones-matmul), normalize on DVE,
  y^T [768,197] = w_proj @ out_allT + b_proj (bias folded into the drain).
Host transposes the gathered y^T back to [B, N, C].

k_bias is structurally zero (reference registers it as a zeros buffer),
so the k-half PSUM drain is a plain wide copy.
"""
import numpy as np
import ml_dtypes

import concourse.bass as bass
import concourse.tile as tile
from concourse import mybir
from concourse.bass_utils import run_bass_kernel_spmd

B, N, C = 128, 197, 768
H, HD = 12, 64
NCORES = 8
BL = B // NCORES          # 16 samples per core
NPAIRS = BL // 2          # 8 sample pairs
N2 = 2 * N                # 394
KT = C // 128             # 6 k-tiles
MT_QK = 2 * C // 128      # 12 m-tiles of qkT
BF16 = mybir.dt.bfloat16
F32 = mybir.dt.float32
AF = mybir.ActivationFunctionType
ALU = mybir.AluOpType
PV_TRAIL = 2
RPB_ON_POOL = True


def _split_sync_waits(nc, max_waits=1, max_updates=1):
    """TPB descriptors have ONE wait and ONE update slot; hoist extras
    onto same-engine NoOps (trailing-nop updates are completion-safe)."""
    n_split = 0
    for f in nc.m.functions:
        for bb in f.blocks:
            old = list(bb.instructions)
            new = []
            changed = False
            for ins in old:
                si = ins.sync_info
                tname = type(ins).__name__
                is_dma = ("TensorLoad" in tname or "TensorSave" in tname
                          or "TensorCopy" in tname or "TriggeredCopy" in tname)
                if si is None or is_dma:
                    new.append(ins)
                    continue
                waits = list(si.on_wait)
                updates = list(si.on_update)
                if len(waits) <= max_waits and len(updates) <= max_updates:
                    new.append(ins)
                    continue
                changed = True
                n_split += 1
                while len(waits) > max_waits:
                    w = waits.pop(0)
                    new.append(mybir.InstNoOp(
                        name=nc.get_next_instruction_name(), engine=ins.engine,
                        sync_info=mybir.SyncInfo(on_wait=[w], on_update=[]),
                        bass_nofuse=True))
                post = []
                while len(updates) > max_updates:
                    u = updates.pop()
                    post.append(mybir.InstNoOp(
                        name=nc.get_next_instruction_name(), engine=ins.engine,
                        sync_info=mybir.SyncInfo(on_wait=[], on_update=[u]),
                        bass_nofuse=True))
                ins.sync_info = mybir.SyncInfo(on_wait=waits, on_update=updates)
                new.append(ins)
                new.extend(post)
            if changed:
                bb.instructions = new
    return n_split


def build_nc(repeat=1, split_waits=True):
    nc = bass.Bass("TRN2")
    xt_d = nc.declare_dram_parameter("xt", [BL, 128, KT * N], BF16, isOutput=False)
    wqkv_d = nc.declare_dram_parameter("wqkv", [128, KT, 3 * C], BF16, isOutput=False)
    wproj_d = nc.declare_dram_parameter("wproj", [128, KT, C], BF16, isOutput=False)
    rpb_d = nc.declare_dram_parameter("rpb", [N, H, N], BF16, isOutput=False)
    qkvb_d = nc.declare_dram_parameter("qkvb", [128, MT_QK, BL], F32, isOutput=False)
    projb_d = nc.declare_dram_parameter("projb", [128, KT, BL], F32, isOutput=False)
    vb_d = nc.declare_dram_parameter("vb", [BL, H * HD], BF16, isOutput=False)
    out_d = nc.declare_dram_parameter("out", [BL, C, N], F32, isOutput=True)

    with tile.TileContext(nc) as tc:
        with (
            tc.tile_pool(name="const", bufs=1) as const,
            tc.tile_pool(name="xtp", bufs=3) as xtp,
            tc.tile_pool(name="qkp", bufs=3) as qkp,
            tc.tile_pool(name="vbp", bufs=4) as vbp,
            tc.tile_pool(name="esp", bufs=8) as esp,
            tc.tile_pool(name="rcp", bufs=5) as rcp,
            tc.tile_pool(name="oap", bufs=3) as oap,
            tc.tile_pool(name="yp", bufs=3) as yp,
            tc.tile_pool(name="psA", bufs=3, space="PSUM") as psA,
            tc.tile_pool(name="psS", bufs=3, space="PSUM") as psS,
            tc.tile_pool(name="psB", bufs=2, space="PSUM") as psB,
        ):
            # ---- resident constants ----
            wqkv_sb = const.tile([128, KT, 3 * C], BF16)
            nc.sync.dma_start(wqkv_sb, wqkv_d[:])
            wproj_sb = const.tile([128, KT, C], BF16)
            nc.sync.dma_start(wproj_sb, wproj_d[:])
            # rpb packed [p, mt, h, n]; rows 69:128 of mt=1 unused
            rpb_ext = const.tile([128, 2, H, N], BF16)
            nc.sync.dma_start(rpb_ext[:, 0], rpb_d[0:128])
            nc.sync.dma_start(rpb_ext[0:69, 1], rpb_d[128:N])
            qkvb_sb = const.tile([128, MT_QK, BL], F32)
            nc.sync.dma_start(qkvb_sb, qkvb_d[:])
            projb_sb = const.tile([128, KT, BL], F32)
            nc.sync.dma_start(projb_sb, projb_d[:])

            # Persistent ping-pong tiles (parity = pair index % 2): the
            # block-diagonal zeros of q2 and the per-head ones blocks of v
            # never change, so write them once instead of every pair.
            q2_pp = []
            for par in range(2):
                q2 = const.tile([128, KT, 2, N2], BF16, tag=f"q2p{par}",
                                name=f"q2p{par}")
                nc.gpsimd.memset(q2[0:64, :, :, N:N2], 0.0)
                nc.gpsimd.memset(q2[64:128, :, :, 0:N], 0.0)
                q2_pp.append(q2)
            # v stationary per head: [v_h | ones64] -> PV matmul output rows
            # 0:64 carry v-weighted sums, rows 64:128 carry the softmax
            # denominator replicated 64x (free partition-broadcast on PE)
            vt_pp = []
            for par in range(2):
                by_s = []
                for s in range(2):
                    by_nt = []
                    for nt in range(2):
                        nts = 128 if nt == 0 else N - 128
                        vt = const.tile([nts, H, 2 * HD], BF16,
                                        tag=f"vt{par}{s}{nt}",
                                        name=f"vt{par}{s}{nt}")
                        nc.gpsimd.memset(vt[:, :, HD:2 * HD], 1.0)
                        by_nt.append(vt)
                    by_s.append(by_nt)
                vt_pp.append(by_s)

            if True:
                NG = repeat * NPAIRS
                st = {}

                def emit_load(p):
                    pp = p % NPAIRS
                    sg = (2 * pp, 2 * pp + 1)
                    xt = xtp.tile([128, KT, N2], BF16, name="xt")
                    for s in range(2):
                        nc.sync.dma_start(
                            xt[:, :, s * N:(s + 1) * N],
                            xt_d[sg[s]].rearrange("p (k n) -> p k n", k=KT))
                    vb_bc = [None, None]
                    for s in range(2):
                        vb_bc[s] = vbp.tile([128, H, HD], BF16, tag="vb", name="vb")
                        nc.scalar.dma_start(
                            vb_bc[s],
                            vb_d[sg[s]:sg[s] + 1, :].rearrange(
                                "o (h d) -> o h d", h=H).to_broadcast([128, H, HD]))
                    qkT = qkp.tile([128, KT, N2], BF16, name="qkT")
                    # q block-diagonal [j2, hp, s, (hh n)]: rows 0:64 carry
                    # [q_h1 | 0], rows 64:128 [0 | q_h2], so each score tile
                    # [S_h1 | S_h2] is ONE full-stream ap394 matmul (short
                    # ap197 matmuls pay a ~173ns PE-SBUF latency floor)
                    st[p] = {"xt": xt, "vb": vb_bc, "qkT": qkT,
                             "q2": q2_pp[p % 2], "v": None}

                def emit_qkv_m(p, m):
                    pp = p % NPAIRS
                    sg0 = 2 * pp
                    xt, qkT, q2 = st[p]["xt"], st[p]["qkT"], st[p]["q2"]
                    ps = psA.tile([128, N2], F32, tag="mm", name="ps")
                    for k in range(KT):
                        nc.tensor.matmul(ps, wqkv_sb[:, k, m * 128:(m + 1) * 128],
                                         xt[:, k, :], start=(k == 0),
                                         stop=(k == KT - 1))
                    if m >= KT:
                        # k rows: bias is structurally zero -> one wide copy
                        nc.scalar.activation(qkT[:, m - KT, :], ps, AF.Copy)
                    else:
                        # q rows: drain + per-sample bias in one DVE op per
                        # diagonal block (bias broadcast along n)
                        for b in range(2):   # head 2m+b -> diagonal block
                            dst = q2[b * 64:(b + 1) * 64, m, :,
                                     b * N:(b + 1) * N]
                            src_ = ps[b * 64:(b + 1) * 64, :].rearrange(
                                "p (s n) -> p s n", s=2)
                            bb = qkvb_sb[b * 64:(b + 1) * 64, m,
                                         sg0:sg0 + 2].to_broadcast([64, 2, N])
                            nc.vector.tensor_add(dst, src_, bb)

                def emit_v_chunk(p, s, nt):
                    xt, vb_bc = st[p]["xt"], st[p]["vb"]
                    if st[p]["v"] is None:
                        st[p]["v"] = [[None, None], [None, None]]
                    nts = 128 if nt == 0 else N - 128
                    vt = vt_pp[p % 2][s][nt]
                    for half in range(2):
                        ps = psA.tile([128, N2], F32, tag="mm", name="ps")
                        for k in range(KT):
                            nc.tensor.matmul(
                                ps[:nts, 0:384],
                                xt[:, k, s * N + nt * 128:
                                   s * N + nt * 128 + nts],
                                wqkv_sb[:, k, 2 * C + half * 384:
                                        2 * C + (half + 1) * 384],
                                start=(k == 0), stop=(k == KT - 1))
                        # drain on Act; v_bias added on Pool (off the
                        # critical path: v is ready an iteration early)
                        nc.scalar.activation(
                            vt[:, half * 6:(half + 1) * 6, 0:HD],
                            ps[:nts, 0:384].rearrange("p (h d) -> p h d", h=6),
                            AF.Copy)
                    nc.gpsimd.tensor_add(
                        vt[:, :, 0:HD], vt[:, :, 0:HD], vb_bc[s][:nts])
                    st[p]["v"][s][nt] = vt

                def emit_v(p):
                    for s in range(2):
                        for nt in range(2):
                            emit_v_chunk(p, s, nt)

                def emit_s(p, s, hp):
                    # merged head-pair score: ONE full-stream ap394 matmul per
                    # m-tile (contraction 128 = stacked head-dims); one PSUM
                    # bank per m-tile so psA can have a third buffer
                    qkT, q2 = st[p]["qkT"], st[p]["q2"]
                    ha = 2 * hp
                    e = esp.tile([128, 2, N2], BF16, tag="es", name="es")
                    for mt in range(2):
                        mts = 128 if mt == 0 else N - 128
                        pss = psS.tile([128, 512], F32, tag="s1", name="pss")
                        nc.tensor.matmul(
                            pss[:mts, 0:N2],
                            qkT[:, hp,
                                s * N + mt * 128: s * N + mt * 128 + mts],
                            q2[:, hp, s, :],
                            start=True, stop=True)
                        nc.scalar.activation(
                            e[0:mts, mt, :], pss[0:mts, 0:N2], AF.Exp)
                        # mt1 rpb on the idle Pool engine; mt0 (bigger, and
                        # first needed by PV) stays on DVE
                        rpb_eng = nc.gpsimd if (RPB_ON_POOL and mt == 1) \
                            else nc.vector
                        rpb_eng.tensor_mul(
                            e[0:mts, mt, :].rearrange("p (b n) -> p b n", b=2),
                            e[0:mts, mt, :].rearrange("p (b n) -> p b n", b=2),
                            rpb_ext[0:mts, mt, ha:ha + 2, :])
                    return e

                def emit_pv(p, s, hp, es, oa):
                    v_sb = st[p]["v"]
                    # stationary [v_h | ones64]: output rows 0:64 = PV sums,
                    # rows 64:128 = softmax denominator replicated 64x (the
                    # broadcast costs nothing on PE; cost = moving free size)
                    pvt = psB.tile([128, N2], F32, tag="pv", name="pvt")
                    for mt in range(2):
                        for hh, h in enumerate((2 * hp, 2 * hp + 1)):
                            # start=True clears has_written for the WHOLE bank:
                            # only the very first matmul may set it.
                            mts = 128 if mt == 0 else N - 128
                            nc.tensor.matmul(
                                pvt[:, hh * N:(hh + 1) * N],
                                v_sb[s][mt][:, h, :],
                                es[0:mts, mt, hh * N:(hh + 1) * N],
                                start=(mt == 0 and hh == 0),
                                stop=(mt == 1 and hh == 1))
                    # normalize: 1/denom via exp(-ln d) on Act -- the DVE
                    # reciprocal instruction measures ~3.1us on real HW for
                    # this shape, vs ~0.45us per Act table op
                    lnd = rcp.tile([64, N2], F32, tag="rc", name="lnd")
                    nc.scalar.activation(lnd[:], pvt[HD:2 * HD, :], AF.Ln)
                    rec_bc = rcp.tile([64, N2], BF16, tag="bc_sb", name="rec_bc")
                    nc.scalar.activation(rec_bc[:], lnd[:], AF.Exp, scale=-1.0)
                    for hh in range(2):
                        nc.vector.tensor_mul(
                            oa[hh * 64:(hh + 1) * 64, hp, s * N:(s + 1) * N],
                            pvt[0:HD, hh * N:(hh + 1) * N],
                            rec_bc[:, hh * N:(hh + 1) * N])

                def emit_proj(p):
                    pp = p % NPAIRS
                    sg = (2 * pp, 2 * pp + 1)
                    oa = st[p]["oa"]
                    for m in range(KT):
                        ps = psA.tile([128, N2], F32, tag="mm", name="ps")
                        for k in range(KT):
                            nc.tensor.matmul(
                                ps, wproj_sb[:, k, m * 128:(m + 1) * 128],
                                oa[:, k, :], start=(k == 0), stop=(k == KT - 1))
                        y = yp.tile([128, N2], F32, tag="y", name="y")
                        for s in range(2):
                            # per-partition bias -> Act can drain+bias
                            nc.scalar.activation(
                                y[:, s * N:(s + 1) * N],
                                ps[:, s * N:(s + 1) * N], AF.Identity,
                                bias=projb_sb[:, m, sg[s]:sg[s] + 1],
                                scale=1.0)
                        for s in range(2):
                            nc.sync.dma_start(
                                out_d[sg[s], m * 128:(m + 1) * 128, :],
                                y[:, s * N:(s + 1) * N])

                # skewed pipeline: pair p attention carries pair p+1 QKV
                emit_load(0)
                for m in range(MT_QK):
                    emit_qkv_m(0, m)
                emit_v(0)
                for p in range(NG):
                    if p + 1 < NG:
                        emit_load(p + 1)
                    oa = oap.tile([128, KT, N2], BF16, name="oa")
                    st[p]["oa"] = oa
                    pending = []
                    iters = [(s, hp) for s in range(2) for hp in range(H // 2)]
                    for i, (s, hp) in enumerate(iters):
                        es = emit_s(p, s, hp)
                        if p + 1 < NG:
                            if i < 8:
                                emit_qkv_m(p + 1, i)
                            else:
                                emit_v_chunk(p + 1, (i - 8) // 2, (i - 8) % 2)
                        pending.append((s, hp, es))
                        if len(pending) > PV_TRAIL:
                            emit_pv(p, *pending.pop(0), oa)
                    for item in pending:
                        emit_pv(p, *item, oa)
                    if p + 1 < NG:
                        for m in range(8, MT_QK):
                            emit_qkv_m(p + 1, m)
                    emit_proj(p)
                    del st[p]
    if split_waits:
        _split_sync_waits(nc)
    return nc


_NC_CACHE = {}


def _get_nc():
    if "nc" not in _NC_CACHE:
        _NC_CACHE["nc"] = build_nc()
    return _NC_CACHE["nc"]


def _prep(x, b_idx, w_qkv, q_bias, k_bias, v_bias, rel_pos_table, rel_index,
          w_proj, b_proj):
    x = np.asarray(x, dtype=np.float32)
    b_idx = np.asarray(b_idx)
    w_qkv = np.asarray(w_qkv, dtype=np.float32)
    q_bias = np.asarray(q_bias, dtype=np.float32)
    k_bias = np.asarray(k_bias, dtype=np.float32)
    v_bias = np.asarray(v_bias, dtype=np.float32)
    rel_pos_table = np.asarray(rel_pos_table, dtype=np.float32)
    rel_index = np.asarray(rel_index)
    w_proj = np.asarray(w_proj, dtype=np.float32)
    b_proj = np.asarray(b_proj, dtype=np.float32)

    scale = HD ** (-0.5)
    # fold attention scale into q weights/bias
    w_all = w_qkv.copy()
    w_all[0:C] *= scale
    wqkvT = np.ascontiguousarray(w_all.T)                      # [C, 3C]
    wqkv_p = wqkvT.reshape(KT, 128, 3 * C).transpose(1, 0, 2)  # [128, KT, 3C]
    wprojT = np.ascontiguousarray(w_proj.T)                    # [C, C]
    wproj_p = wprojT.reshape(KT, 128, C).transpose(1, 0, 2)    # [128, KT, C]

    # per-sample gathered biases
    qk_bias = np.concatenate([q_bias * scale, k_bias], axis=1)[b_idx]  # [B, 2C]
    qkvb_all = qk_bias.T.reshape(MT_QK, 128, B)                # [12, 128, B]
    projb_all = b_proj[b_idx].T.reshape(KT, 128, B)            # [6, 128, B]
    vb_all = v_bias[b_idx]                                     # [B, C]

    # exp of transposed relative-position bias: rpbT[m, h, n] = rpb[h][n, m]
    tbl = rel_pos_table[rel_index.reshape(-1)].reshape(N, N, H)  # [n, m, h]
    rpbT = np.exp(tbl.transpose(1, 2, 0))                        # [m, h, n]
    rpb_p = np.ascontiguousarray(rpbT, dtype=np.float32).astype(ml_dtypes.bfloat16)

    # x^T packed: [B, 128, KT*N] with partition p = c % 128, free (k, n)
    xT = x.transpose(0, 2, 1)                                  # [B, C, N]
    xt_p = xT.reshape(B, KT, 128, N).transpose(0, 2, 1, 3).reshape(B, 128, KT * N)
    xt_p = xt_p.astype(ml_dtypes.bfloat16)

    wqkv_p = np.ascontiguousarray(wqkv_p).astype(ml_dtypes.bfloat16)
    wproj_p = np.ascontiguousarray(wproj_p).astype(ml_dtypes.bfloat16)

    in_maps = []
    for i in range(NCORES):
        lo, hi = i * BL, (i + 1) * BL
        in_maps.append({
            "xt": np.ascontiguousarray(xt_p[lo:hi]),
            "wqkv": wqkv_p,
            "wproj": wproj_p,
            "rpb": rpb_p,
            "qkvb": np.ascontiguousarray(qkvb_all.transpose(1, 0, 2)[:, :, lo:hi]).astype(np.float32),
            "projb": np.ascontiguousarray(projb_all.transpose(1, 0, 2)[:, :, lo:hi]).astype(np.float32),
            "vb": np.ascontiguousarray(vb_all[lo:hi]).astype(ml_dtypes.bfloat16),
        })

    return in_maps


def _gather(results):
    outT = np.concatenate([results[i]["out"] for i in range(NCORES)], axis=0)
    return np.ascontiguousarray(outT.transpose(0, 2, 1))


def kernel(**inputs):
    in_maps = _prep(**inputs)
    nc = _get_nc()
    res = run_bass_kernel_spmd(nc, in_maps, list(range(NCORES))).results
    return _gather(res)

